# revision 5
# baseline (speedup 1.0000x reference)
"""Conv2d 3x3 (stride 1, pad 1) forward on 8 Trainium2 NeuronCores.

v4: fold the 3 kw-taps into the matmul contraction dim (K=96) so each
round needs 3 passes instead of 9, and run the 2 images of a pair
concurrently on the two column halves of the PE array.

Per-core design (4 images = 2 pairs):
  - xr{p} tile [128, 2img, 130, 130] bf16 holds pair p.  Partition
    group g (32 parts) holds replica g: rep_r[ci, row, c] =
    xpad[ci, row, c+r] (column-shifted).  rep0 comes from HBM via DMA;
    rep1/rep2 are VectorE shifted copies (bf16 SBUF->SBUF).
      pair0: rep0@0-31  rep1@32-63  rep2@64-95   (even DMA ports)
      pair1: rep1@0-31  rep2@32-63  rep0@64-95   (odd DMA ports)
  - Per round (4 output rows) and pair: 3 matmuls (kh=0..2) of
    K=96 x N=512 accumulate in PSUM [128, 4, 128]; the two images go
    to column tiles (0,0) / (0,64) and drain as one 128-partition op.
  - Weight stationary [96, 64] per (pair, kh); pair1's rows are
    permuted to match its replica order.
  - Drain: ScalarE (activation+bias) for pair0 and every other pair1
    round; VectorE (tensor_scalar_add) for the rest.  bf16 staging,
    output DMA batched 4 rounds deep (2 MiB per store).
"""
import sys
sys.path.insert(0, '/opt/trn_rl_repo')
import numpy as np
import ml_dtypes

BF16 = ml_dtypes.bfloat16
B, Cin, H, W = 32, 32, 128, 128
Cout, KH, KW = 64, 3, 3
NCORES = 8
BPC = B // NCORES          # images per core
NPAIR = 2                  # image pairs per core
Hp, Wp = H + 2, W + 2
R = 4                      # output rows per round
NROUND = H // R
RB = 4                     # rounds per output DMA batch
NCH = 8                    # input DMA chunks
ROWS_CH = H // NCH
PERM1 = (1, 2, 0)          # pair1 partition-group -> kw

_cache = {}


def _build_program():
    from concourse import bacc
    import concourse.mybir as mybir
    from concourse.tile import TileContext

    f32 = mybir.dt.float32
    bf16 = mybir.dt.bfloat16
    Act = mybir.ActivationFunctionType

    nc = bacc.Bacc("TRN2", target_bir_lowering=False, debug=False,
                   num_devices=NCORES)
    x_ext = nc.declare_dram_parameter("x", [NPAIR, Cin, 2, H, W], bf16,
                                      isOutput=False)
    w_ext = nc.declare_dram_parameter("w", [128, NPAIR, KH, Cout], bf16,
                                      isOutput=False)
    b_ext = nc.declare_dram_parameter("b", [128, 1], f32, isOutput=False)
    out_ext = nc.declare_dram_parameter("out", [BPC * Cout, H, W], bf16,
                                        isOutput=True)

    with TileContext(nc) as tc:
        with tc.tile_pool(name="xr", bufs=1) as xpool, \
             tc.tile_pool(name="const", bufs=1) as cpool, \
             tc.tile_pool(name="stage", bufs=2) as opool, \
             tc.tile_pool(name="psum", bufs=8, space="PSUM") as ppool:

            xr = [xpool.tile([128, 2, Hp, Wp], bf16, name=f"xr{p}")
                  for p in range(NPAIR)]
            wt = cpool.tile([128, NPAIR, KH, Cout], bf16)
            bt = cpool.tile([128, 1], f32)

            nc.sync.dma_start(out=wt[:], in_=w_ext[:])
            nc.sync.dma_start(out=bt[:], in_=b_ext[:])

            rep0 = (0, 64)          # rep0 partition base per pair
            # zero the rep0 halo; shifted copies propagate it
            for p in range(NPAIR):
                b0 = rep0[p]
                nc.vector.memset(xr[p][b0:b0 + 32, :, 0, :], 0.0)
                nc.vector.memset(xr[p][b0:b0 + 32, :, Hp - 1, :], 0.0)
                nc.vector.memset(xr[p][b0:b0 + 32, :, :, 0], 0.0)
                nc.vector.memset(xr[p][b0:b0 + 32, :, :, Wp - 1], 0.0)

            for g in range(NCH):
                r0 = g * ROWS_CH
                for p in range(NPAIR):
                    b0 = rep0[p]
                    for im in range(2):
                        nc.sync.dma_start(
                            out=xr[p][b0:b0 + 32, im,
                                      1 + r0:1 + r0 + ROWS_CH, 1:1 + W],
                            in_=x_ext[p, :, im, r0:r0 + ROWS_CH, :])
                # replica rows for this chunk (include halo rows at ends)
                ra = 0 if g == 0 else 1 + r0
                rb_ = Hp if g == NCH - 1 else 1 + r0 + ROWS_CH
                # pair0: rep1 @32 (shift 1), rep2 @64 (shift 2)
                nc.vector.tensor_copy(
                    xr[0][32:64, :, ra:rb_, 0:W],
                    xr[0][0:32, :, ra:rb_, 1:1 + W])
                nc.vector.tensor_copy(
                    xr[0][64:96, :, ra:rb_, 0:W],
                    xr[0][0:32, :, ra:rb_, 2:2 + W])
                # pair1: rep1 @0 (shift 1), rep2 @32 (shift 2)
                nc.vector.tensor_copy(
                    xr[1][0:32, :, ra:rb_, 0:W],
                    xr[1][64:96, :, ra:rb_, 1:1 + W])
                nc.vector.tensor_copy(
                    xr[1][32:64, :, ra:rb_, 0:W],
                    xr[1][64:96, :, ra:rb_, 2:2 + W])

            out_v = out_ext.rearrange(
                "(ip half co) h w -> (half co) ip (h w)",
                ip=2, half=2, co=Cout)

            for k in range(NROUND):
                h0 = k * R
                if k % RB == 0:
                    ost = opool.tile([128, 2, RB * R, W], bf16, tag="ost")
                roff = (k % RB) * R
                for p in range(NPAIR):
                    ps = ppool.tile([128, R, W], f32, tag="ps",
                                    name=f"ps{k}_{p}")
                    for kh in range(KH):
                        for half in range(2):
                            nc.tensor.matmul(
                                ps[64 * half:64 * half + 64, :, :],
                                wt[0:96, p, kh, :],
                                xr[p][0:96, half, h0 + kh:h0 + kh + R, 0:W],
                                start=(kh == 0), stop=(kh == KH - 1),
                                tile_position=(0, 64 * half))
                    if p == 0 or k % 2 == 1:
                        nc.scalar.activation(ost[:, p, roff:roff + R, :],
                                             ps[:, :, :], Act.Identity,
                                             bias=bt[:, :])
                    else:
                        nc.vector.tensor_scalar_add(
                            ost[:, p, roff:roff + R, :], ps[:, :, :],
                            bt[:, :])
                if k % RB == RB - 1:
                    hb = (k - (RB - 1)) * R
                    nc.sync.dma_start(
                        out=out_v[:, :, hb * W:(hb + RB * R) * W],
                        in_=ost[:, :, :, :])

    nc.compile()
    return nc


def _get_program():
    if "nc" not in _cache:
        _cache["nc"] = _build_program()
    return _cache["nc"]


def _prep_inputs(x, kernel, bias):
    # weights -> [32*g + ci, pair, kh, co]; pair1's replica groups are
    # permuted (rep1, rep2, rep0) so its group g holds kw = PERM1[g]
    kr = kernel.reshape(Cout, Cin, KH, KW).astype(np.float32)
    w = np.zeros((128, NPAIR, KH, Cout), dtype=np.float32)
    for g in range(3):
        # [ci, kh, co]
        w[32 * g:32 * g + 32, 0] = np.transpose(kr[:, :, :, g], (1, 2, 0))
        w[32 * g:32 * g + 32, 1] = np.transpose(kr[:, :, :, PERM1[g]],
                                                (1, 2, 0))
    w = np.ascontiguousarray(w).astype(BF16)
    bvec = np.tile(bias.astype(np.float32), 2)[:, None]
    b = np.ascontiguousarray(bvec)
    xb = x.astype(BF16)
    in_maps = []
    for c in range(NCORES):
        # [4, 32, 128, 128] -> [pair, ci, img, h, w]
        xs = xb[c * BPC:(c + 1) * BPC].reshape(NPAIR, 2, Cin, H, W)
        xs = np.ascontiguousarray(np.transpose(xs, (0, 2, 1, 3, 4)))
        in_maps.append({"x": xs, "w": w, "b": b})
    return in_maps


def _run(inputs, trace=False):
    from concourse.bass_utils import run_bass_kernel_spmd
    nc = _get_program()
    in_maps = _prep_inputs(inputs["x"], inputs["kernel"], inputs["bias"])
    res = run_bass_kernel_spmd(nc, in_maps, list(range(NCORES)), trace=trace)
    out = np.concatenate(
        [res.results[c]["out"].reshape(BPC, Cout, H, W)
         for c in range(NCORES)], axis=0)
    return out.astype(np.float32), res


def kernel(**inputs):
    out, _ = _run(inputs, trace=False)
    return out
